# revision 5
# baseline (speedup 1.0000x reference)
"""Self-contained TRN2 Bass kernel: single-head encoder self-attention, v3.

kernel(**inputs) takes the FULL unsharded inputs and returns the full
[2, 4096, 128] fp32 output of

    out = softmax((X Wq)(X Wk)^T / sqrt(128), axis=keys) @ (X Wv)

(the reference's query-axis mask is a softmax no-op; masks and the
encoder_output_embedding inputs are unused).

Sharding: sequence-parallel over queries; core c handles batch c//4,
query rows (c%4)*1024 .. +1024, with keys/values of the full 4096-row
batch sequence recomputed per core.

v3 design (v1 baseline ~116us, v2 ~measured half of that):
  - Weight folding on host: M = Wq Wk^T / sqrt(D): scores = (X M) X^T.
  - Host sends X^T in fp16; all matmuls 16-bit (fp32r stationary
    reloads cost ~1.4us on TRN2, 16-bit reloads are free).
  - NO exact row-max pass. The softmax bias only needs to be within
    ~+-80 of the true row max when P is stored in bf16 (range e+-88):
    any such bias yields the mathematically identical normalized
    result. Host picks the 32 keys with extreme row-sums u = X.1 (the
    scores' dominant rank-1 direction); the device computes exact
    scores against just those 32 candidate keys (one tiny matmul per
    128-query sub-block) and biases by candidate_max + 55. The
    candidate max is a subset max, so est <= rowmax always (dominant
    probs stay >= e^-55, far above bf16's e^-87 floor), and on this
    data rowmax - est <= 38 (overflow would need > 143). This removes
    the entire second scores pass and ~43us of DVE PSUM max-reduces.
  - exp in 2048-wide chunks on ACT (amortizes PSUM access + accum
    read) writing P bf16 + row-sum l via accum_out.
  - P^T via DMA XBAR transpose (14ns per 16x128 tile, on the
    otherwise idle DMA engines): no PE transposes of P, no DVE copies.
  - V computed directly in keys-on-partition chunks (lhsT = X^T chunk
    stationary): no V transposes. P.V accumulates per 2-sub-block
    group; groups drain progressively through the scores phase so the
    tail is one small group.
  - Output written as fp16 (host casts up; ~2e-4 extra error).
"""
import contextlib

import numpy as np

import concourse.tile as tile
from concourse.tile import add_dep_helper
from concourse import bacc, mybir
from concourse.bass_utils import run_bass_kernel_spmd

F32 = mybir.dt.float32
F16 = mybir.dt.float16
BF16 = mybir.dt.bfloat16
AX = mybir.AxisListType
ALU = mybir.AluOpType
ACTF = mybir.ActivationFunctionType

D = 128
B_SZ = 2
S_SRC = 4096
N_CORES = 8
N_ROWS = (B_SZ * S_SRC) // N_CORES  # 1024 query rows per core
N_CAND = 32
MARGIN = 55.0

_NC_CACHE = {}


def _make_identity(nc, ap):
    nc.gpsimd.memset(ap, 0.0)
    nc.gpsimd.affine_select(
        out=ap, in_=ap, compare_op=ALU.not_equal, fill=1.0, base=0,
        pattern=[[-1, ap.shape[1]]], channel_multiplier=1)


def _build_attn(n_tok=S_SRC, n_rows=N_ROWS, n_cores=N_CORES, m_repeat=None):
    nsb = n_rows // 128          # 8 query sub-blocks of 128 rows
    tch = n_tok // 128           # 32 key chunks of 128 (PV granularity)
    spg = 2                      # sub-blocks per PV group
    ngr = nsb // spg             # PV groups

    nc = bacc.Bacc("TRN2", target_bir_lowering=False, debug=False,
                   num_devices=n_cores)
    xt_d = nc.dram_tensor("xt", [D, n_tok], F16, kind="ExternalInput")
    m_d = nc.dram_tensor("m", [D, D], F16, kind="ExternalInput")
    wv_d = nc.dram_tensor("wv", [D, D], F16, kind="ExternalInput")
    xc_d = nc.dram_tensor("xc", [D, N_CAND], F16, kind="ExternalInput")
    out_d = nc.dram_tensor("out", [(n_rows // 256) * D, 256], F32,
                           kind="ExternalOutput")
    l_d = nc.dram_tensor("lrec", [D, 8], F32, kind="ExternalOutput")

    with tile.TileContext(nc) as tc:
        with tc.tile_pool(name="const", bufs=1) as constp, \
             tc.tile_pool(name="big", bufs=1) as bigp, \
             tc.tile_pool(name="pbuf", bufs=5) as pbufp, \
             tc.tile_pool(name="ptsb", bufs=3) as ptsbp, \
             tc.tile_pool(name="sm", bufs=10) as smp, \
             tc.tile_pool(name="osb", bufs=2) as osbp, \
             tc.tile_pool(name="psB", bufs=2, space="PSUM") as psB, \
             tc.tile_pool(name="psE", bufs=1, space="PSUM") as psE, \
             tc.tile_pool(name="psV", bufs=3, space="PSUM") as psV:

            # ---- constants ----
            m_t = constp.tile([D, D], F16, tag="m")
            wv_t = constp.tile([D, D], F16, tag="wv")
            xc_t = constp.tile([D, N_CAND], F16, tag="xc")
            nc.sync.dma_start(out=m_t[:], in_=m_d.ap())
            id_f = constp.tile([D, D], F32, tag="id_f")
            _make_identity(nc, id_f[:])
            id_b = constp.tile([D, D], BF16, tag="id_b")
            nc.vector.tensor_copy(id_b[:], id_f[:])
            warm = constp.tile([128, 1], F32, tag="warm")
            nc.scalar.activation(warm[:], id_f[:, 0:1], ACTF.Exp)

            rep_ctx = tc.For_i(0, m_repeat, 1) if m_repeat else \
                contextlib.nullcontext()
            rep_ctx.__enter__()

            xT = bigp.tile([D, n_tok], F16, tag="xT")
            aT = bigp.tile([D, n_rows], F16, tag="aT")
            v_b = bigp.tile([D, tch, D], BF16, tag="v_b")  # V [j, k, dv]

            nc.sync.dma_start(out=xT[:, 0:1024], in_=xt_d.ap()[:, 0:1024])
            nc.sync.dma_start(out=xc_t[:], in_=xc_d.ap())
            nc.sync.dma_start(out=wv_t[:], in_=wv_d.ap())
            for c0 in range(1024, n_tok, 1024):
                nc.sync.dma_start(out=xT[:, c0:c0 + 1024],
                                  in_=xt_d.ap()[:, c0:c0 + 1024])

            # ---- A^T = M^T Xq^T (scores stationary), fp16 ----
            for h in range(n_rows // 512):
                pa = psB.tile([128, 2, 512], F32, tag="psB",
                              name=f"paT{h}")[:, 0, :]
                nc.tensor.matmul(pa[:], lhsT=m_t[:],
                                 rhs=xT[:, h * 512:(h + 1) * 512],
                                 start=True, stop=True)
                if h == 0:
                    # first 128 cols first: unblocks est(0) + sub-block 0
                    nc.vector.tensor_copy(aT[:, 0:128], pa[0:128, 0:128])
                    nc.vector.tensor_copy(aT[:, 128:512], pa[:, 128:512])
                else:
                    nc.vector.tensor_copy(aT[:, h * 512:(h + 1) * 512], pa[:])

            # ---- V chunks directly keys-on-partition (bf16); emitted in
            # slices from the main loop (s=0/1 PV slots) ----
            def v_proj_slice(b0):
                pv = psV.tile([128, 4, D], F32, tag="psV",
                              name=f"pv{b0}")
                for j in range(4):
                    nc.tensor.matmul(
                        pv[:, j, :],
                        lhsT=xT[:, (b0 + j) * D:(b0 + j + 1) * D],
                        rhs=wv_t[:], start=True, stop=True)
                nc.vector.tensor_copy(
                    v_b[:, b0:b0 + 4, :].rearrange("p a b -> p (a b)"),
                    pv[:].rearrange("p a b -> p (a b)"))

            negms, lsums, ptsbs, pbufs, l2s, ppvs = {}, {}, {}, {}, {}, {}
            lall = bigp.tile([128, nsb], F32, tag="lall")
            smm_log, pv_pending = [], []

            def _resolve_pv_deps():
                for ent in pv_pending[:]:
                    inst, tgt = ent
                    if tgt < len(smm_log):
                        add_dep_helper(inst.ins, smm_log[tgt].ins,
                                       sync=False,
                                       reason="pv_after_future_smm")
                        pv_pending.remove(ent)

            def est(s):
                """negm(s) = -(max over candidate keys + MARGIN)."""
                ctx = tc.high_priority(offset=500000)
                ctx.__enter__()
                r0 = s * 128
                pe_ = psE.tile([128, N_CAND], F32, tag="psE")
                nc.tensor.matmul(pe_[:], lhsT=aT[:, r0:r0 + 128],
                                 rhs=xc_t[:], start=True, stop=True)
                nraw = smp.tile([128, 1], F32, tag="nraw")
                nc.vector.tensor_reduce(nraw[:], pe_[:],
                                        axis=AX.X, op=ALU.max, negate=True)
                negm = smp.tile([128, 1], F32, tag="negm")
                nc.vector.tensor_scalar_add(negm[:], nraw[:], -MARGIN)
                negms[s] = negm
                ctx.__exit__(None, None, None)

            def pass2_quarter(s, q):
                ctx = tc.high_priority(offset=500000)
                ctx.__enter__()
                r0 = s * 128
                si = s % spg
                if q == 0:
                    pbufs[s] = pbufp.tile([128, n_tok], BF16, tag="p_s",
                                          name=f"p_s{s}")
                    l2s[s] = smp.tile([128, 4], F32, tag="l2",
                                      name=f"l2_{s}")
                    if si == 0:
                        ptsbs[s // spg] = ptsbp.tile(
                            [128, tch, spg, 128], BF16, tag="ptsb",
                            name=f"ptg{s // spg}")
                ps = pbufs[s]
                pt = ptsbs[s // spg]
                pb = psB.tile([128, 2, 512], F32, tag="psB")
                for cc in range(2):
                    k0 = (q * 2 + cc) * 512
                    mm = nc.tensor.matmul(pb[:, cc, :],
                                          lhsT=aT[:, r0:r0 + 128],
                                          rhs=xT[:, k0:k0 + 512],
                                          start=True, stop=True)
                    if cc == 0:
                        smm_log.append(mm)
                _resolve_pv_deps()
                h0 = q * 1024
                if q < 2 or s >= nsb - 2:
                    # ACT accumulates l for the first half (279ns/instr
                    # accumulator read); DVE row-sums the second half from
                    # P in SBUF off the critical path (engine balance).
                    nc.scalar.activation(
                        ps[:, h0:h0 + 1024],
                        pb[:].rearrange("p a b -> p (a b)"),
                        ACTF.Exp, bias=negms[s][:, 0:1],
                        accum_out=l2s[s][:, q:q + 1])
                else:
                    nc.scalar.activation(
                        ps[:, h0:h0 + 1024],
                        pb[:].rearrange("p a b -> p (a b)"),
                        ACTF.Exp, bias=negms[s][:, 0:1])
                if q % 2 == 1:
                    half = q // 2
                    if s == nsb - 1:
                        for j4 in range(4):
                            ptp = psV.tile([128, 4, 128], BF16, tag="psV",
                                           name=f"ptp{half}_{j4}")
                            for jj in range(4):
                                j = half * 16 + j4 * 4 + jj
                                nc.tensor.transpose(
                                    ptp[:, jj, :], ps[:, j * 128:(j + 1) * 128],
                                    id_b[:])
                            nc.vector.tensor_copy(
                                pt[:, half * 16 + j4 * 4:
                                   half * 16 + j4 * 4 + 4, si, :],
                                ptp[:])
                    else:
                        nc.sync.dma_start_transpose(
                            out=pt[:, half * 16:(half + 1) * 16, si, :],
                            in_=ps[:, half * 2048:(half + 1) * 2048])
                if q == 3:
                    lsum = smp.tile([128, 1], F32, tag="lsum")
                    if s >= nsb - 2:
                        # tail sub-blocks: l fully from ACT accum (keeps the
                        # DVE reduce off the drain path)
                        nc.vector.tensor_reduce(lsum[:], l2s[s][:],
                                                axis=AX.X, op=ALU.add)
                    else:
                        ldve = smp.tile([128, 1], F32, tag="ldve")
                        nc.vector.tensor_reduce(ldve[:], ps[:, 2048:4096],
                                                axis=AX.X, op=ALU.add)
                        lacc = smp.tile([128, 1], F32, tag="lacc")
                        nc.vector.tensor_reduce(lacc[:], l2s[s][:, 0:2],
                                                axis=AX.X, op=ALU.add)
                        nc.vector.tensor_tensor(out=lsum[:], in0=lacc[:],
                                                in1=ldve[:], op=ALU.add)
                    nc.vector.reciprocal(lall[:, s:s + 1], lsum[:])
                ctx.__exit__(None, None, None)

            def group_pv(g, j0, j1):
                if j0 == 0:
                    ppvs[g] = psV.tile([D, spg * 128], F32, tag="psV",
                                       name=f"ppv{g}")
                ppv = ppvs[g]
                ptg = ptsbs[g]
                for j in range(j0, j1):
                    pvmm = nc.tensor.matmul(
                        ppv[:],
                        lhsT=v_b[:, j, :],
                        rhs=ptg[:, j, :, :].rearrange("p a b -> p (a b)"),
                        start=(j == 0), stop=(j == tch - 1))
                    pv_pending.append((pvmm, len(smm_log) - 1 + 3))

            def group_out(g):
                rw = spg * 128
                ppv = ppvs[g]
                osb = osbp.tile([D, rw], F32, tag="osb")
                nc.vector.tensor_copy(osb[:], ppv[:])
                nc.sync.dma_start(out=out_d.ap()[g * D:(g + 1) * D, :],
                                  in_=osb[:])
                if g == ngr - 1:
                    nc.sync.dma_start(out=l_d.ap(), in_=lall[:])

            # ACT-paced pipeline: est(s+1) + PV slices fill the TE gaps
            # behind each exp; PV groups of 2 sub-blocks drain progressively.
            # per sub-block: 4 quarters (2 S-matmuls + one 1024-wide exp
            # each, psB double-buffered so the exp chain never stalls);
            # est(s+1) and 4-j PV slices of the draining group slot between
            # quarters to keep the PE fed.
            # PV slice schedule: each group's 32 j-chunks run as four 8-j
            # slices starting a full 2 quarters after its last xbar issues
            # (xbar exec lags its exp by ~3-4us: dispatch + DGE delay +
            # queue), in chunkier bursts that keep the PE p-state up.
            sched = {}
            for g_ in range(ngr - 1):
                b_ = 2 * g_ + 2
                sched.setdefault((b_, 2), []).append((g_, 0, 8))
                sched.setdefault((b_, 3), []).append((g_, 8, 16))
                sched.setdefault((b_ + 1, 0), []).append((g_, 16, 24))
                sched.setdefault((b_ + 1, 1), []).append((g_, 24, 32))
            # last group: half-A comes from PE transposes (no DMA), drain in
            # the final sub-block's back quarters
            sched.setdefault((nsb - 1, 2), []).append((ngr - 1, 0, 8))
            sched.setdefault((nsb - 1, 3), []).append((ngr - 1, 8, 16))

            est(0)
            for s in range(nsb):
                for q in range(4):
                    pass2_quarter(s, q)
                    if q == 0 and s + 1 < nsb:
                        est(s + 1)
                    for g_, j0_, j1_ in sched.get((s, q), []):
                        group_pv(g_, j0_, j1_)
                    if s < 2:
                        v_proj_slice((s * 4 + q) * 4)
                if s % 2 == 1 and s >= 3:
                    group_out((s - 3) // 2)
            group_pv(ngr - 1, 16, 32)
            group_out(ngr - 1)

            rep_ctx.__exit__(None, None, None)
    nc.compile()
    return nc


def _get_nc():
    if "nc" not in _NC_CACHE:
        _NC_CACHE["nc"] = _build_attn()
    return _NC_CACHE["nc"]


def _in_maps(input_embeddings, w_query, w_key, w_value):
    d = input_embeddings.shape[-1]
    m_fused = ((w_query.astype(np.float64) @ w_key.astype(np.float64).T)
               / np.sqrt(d)).astype(np.float16)
    wv16 = w_value.astype(np.float16)
    shards_per_b = N_CORES // B_SZ
    maps = []
    for c in range(N_CORES):
        b, s = divmod(c, shards_per_b)
        xb = input_embeddings[b]
        xt_full = xb.astype(np.float16).T                    # [128, 4096]
        # key permutation: this core's own query rows first, so the
        # query slice is xt[:, :N_ROWS] (softmax/PV are key-perm invariant)
        q0 = s * N_ROWS
        xt = np.ascontiguousarray(np.concatenate(
            [xt_full[:, q0:q0 + N_ROWS], xt_full[:, :q0],
             xt_full[:, q0 + N_ROWS:]], axis=1))
        u = xb.sum(axis=1)
        idx = np.argsort(u)
        cand = np.concatenate([idx[:N_CAND // 2], idx[-N_CAND // 2:]])
        xc = np.ascontiguousarray(xt_full[:, cand])          # [128, 32]
        maps.append({"xt": xt, "m": m_fused, "wv": wv16, "xc": xc})
    return maps


def kernel(input_embeddings, token_attention_masks_source=None,
           token_attention_masks_target=None, encoder_output_embedding=None,
           w_query=None, w_key=None, w_value=None, **_unused):
    """Full inputs in, full output out. Runs on 8 NeuronCores (SPMD)."""
    input_embeddings = np.asarray(input_embeddings, dtype=np.float32)
    b_sz, s_src, d = input_embeddings.shape
    assert (b_sz, s_src, d) == (B_SZ, S_SRC, D), "kernel compiled for 2x4096x128"

    maps = _in_maps(input_embeddings,
                    np.asarray(w_query, dtype=np.float32),
                    np.asarray(w_key, dtype=np.float32),
                    np.asarray(w_value, dtype=np.float32))
    res = run_bass_kernel_spmd(_get_nc(), maps, list(range(N_CORES)))
    out = np.empty((B_SZ, S_SRC, D), np.float32)
    shards_per_b = N_CORES // B_SZ
    for c in range(N_CORES):
        b, s = divmod(c, shards_per_b)
        ot = res.results[c]["out"].astype(np.float32).reshape(4, D, 256)
        lr = res.results[c]["lrec"].astype(np.float32)  # [128, 8]
        rows = ot.transpose(0, 2, 1).reshape(N_ROWS, D)
        lrec = lr.T.reshape(N_ROWS, 1)
        out[b, s * N_ROWS:(s + 1) * N_ROWS] = rows * lrec
    return out


# revision 6
# speedup vs baseline: 1.3796x; 1.3796x over previous
"""Self-contained TRN2 Bass kernel: single-head encoder self-attention, v3.

kernel(**inputs) takes the FULL unsharded inputs and returns the full
[2, 4096, 128] fp32 output of

    out = softmax((X Wq)(X Wk)^T / sqrt(128), axis=keys) @ (X Wv)

(the reference's query-axis mask is a softmax no-op; masks and the
encoder_output_embedding inputs are unused).

Sharding: sequence-parallel over queries; core c handles batch c//4,
query rows (c%4)*1024 .. +1024, with keys/values of the full 4096-row
batch sequence recomputed per core.

v3 design (v1 baseline ~116us, v2 ~measured half of that):
  - Weight folding on host: M = Wq Wk^T / sqrt(D): scores = (X M) X^T.
  - Host sends X^T in fp16; all matmuls 16-bit (fp32r stationary
    reloads cost ~1.4us on TRN2, 16-bit reloads are free).
  - NO exact row-max pass. The softmax bias only needs to be within
    ~+-80 of the true row max when P is stored in bf16 (range e+-88):
    any such bias yields the mathematically identical normalized
    result. Host picks the 32 keys with extreme row-sums u = X.1 (the
    scores' dominant rank-1 direction); the device computes exact
    scores against just those 32 candidate keys (one tiny matmul per
    128-query sub-block) and biases by candidate_max + 55. The
    candidate max is a subset max, so est <= rowmax always (dominant
    probs stay >= e^-55, far above bf16's e^-87 floor), and on this
    data rowmax - est <= 38 (overflow would need > 143). This removes
    the entire second scores pass and ~43us of DVE PSUM max-reduces.
  - exp in 2048-wide chunks on ACT (amortizes PSUM access + accum
    read) writing P bf16 + row-sum l via accum_out.
  - P^T via DMA XBAR transpose (14ns per 16x128 tile, on the
    otherwise idle DMA engines): no PE transposes of P, no DVE copies.
  - V computed directly in keys-on-partition chunks (lhsT = X^T chunk
    stationary): no V transposes. P.V accumulates per 2-sub-block
    group; groups drain progressively through the scores phase so the
    tail is one small group.
  - Output written as fp16 (host casts up; ~2e-4 extra error).
"""
import contextlib

import numpy as np

import concourse.tile as tile
from concourse.tile import add_dep_helper
from concourse import bacc, mybir
from concourse.bass_utils import run_bass_kernel_spmd

F32 = mybir.dt.float32
F16 = mybir.dt.float16
BF16 = mybir.dt.bfloat16
AX = mybir.AxisListType
ALU = mybir.AluOpType
ACTF = mybir.ActivationFunctionType

D = 128
B_SZ = 2
S_SRC = 4096
N_CORES = 8
N_ROWS = (B_SZ * S_SRC) // N_CORES  # 1024 query rows per core
N_CAND = 32
MARGIN = 55.0

_NC_CACHE = {}


def _make_identity(nc, ap):
    nc.gpsimd.memset(ap, 0.0)
    nc.gpsimd.affine_select(
        out=ap, in_=ap, compare_op=ALU.not_equal, fill=1.0, base=0,
        pattern=[[-1, ap.shape[1]]], channel_multiplier=1)


def _build_attn(n_tok=S_SRC, n_rows=N_ROWS, n_cores=N_CORES, m_repeat=None):
    nsb = n_rows // 128          # 8 query sub-blocks of 128 rows
    tch = n_tok // 128           # 32 key chunks of 128 (PV granularity)
    spg = 2                      # sub-blocks per PV group
    ngr = nsb // spg             # PV groups

    nc = bacc.Bacc("TRN2", target_bir_lowering=False, debug=False,
                   num_devices=n_cores)
    xt_d = nc.dram_tensor("xt", [D, n_tok], F16, kind="ExternalInput")
    m_d = nc.dram_tensor("m", [D, D], F16, kind="ExternalInput")
    wv_d = nc.dram_tensor("wv", [D, D], F16, kind="ExternalInput")
    xc_d = nc.dram_tensor("xc", [D, N_CAND], F16, kind="ExternalInput")
    out_d = nc.dram_tensor("out", [(n_rows // 256) * D, 256], F32,
                           kind="ExternalOutput")
    l_d = nc.dram_tensor("lrec", [D, 8], F32, kind="ExternalOutput")

    with tile.TileContext(nc) as tc:
        with tc.tile_pool(name="const", bufs=1) as constp, \
             tc.tile_pool(name="big", bufs=1) as bigp, \
             tc.tile_pool(name="pbuf", bufs=5) as pbufp, \
             tc.tile_pool(name="ptsb", bufs=3) as ptsbp, \
             tc.tile_pool(name="sm", bufs=10) as smp, \
             tc.tile_pool(name="osb", bufs=2) as osbp, \
             tc.tile_pool(name="psB", bufs=2, space="PSUM") as psB, \
             tc.tile_pool(name="psE", bufs=1, space="PSUM") as psE, \
             tc.tile_pool(name="psV", bufs=3, space="PSUM") as psV:

            # ---- constants ----
            m_t = constp.tile([D, D], F16, tag="m")
            wv_t = constp.tile([D, D], F16, tag="wv")
            xc_t = constp.tile([D, N_CAND], F16, tag="xc")
            nc.sync.dma_start(out=m_t[:], in_=m_d.ap())
            id_f = constp.tile([D, D], F32, tag="id_f")
            _make_identity(nc, id_f[:])
            id_b = constp.tile([D, D], BF16, tag="id_b")
            nc.vector.tensor_copy(id_b[:], id_f[:])
            warm = constp.tile([128, 1], F32, tag="warm")
            nc.scalar.activation(warm[:], id_f[:, 0:1], ACTF.Exp)

            rep_ctx = tc.For_i(0, m_repeat, 1) if m_repeat else \
                contextlib.nullcontext()
            rep_ctx.__enter__()

            xT = bigp.tile([D, n_tok], F16, tag="xT")
            aT = bigp.tile([D, n_rows], F16, tag="aT")
            v_b = bigp.tile([D, tch, D], BF16, tag="v_b")  # V [j, k, dv]

            nc.sync.dma_start(out=xT[:, 0:1024], in_=xt_d.ap()[:, 0:1024])
            nc.sync.dma_start(out=xc_t[:], in_=xc_d.ap())
            nc.sync.dma_start(out=wv_t[:], in_=wv_d.ap())
            for c0 in range(1024, n_tok, 1024):
                nc.sync.dma_start(out=xT[:, c0:c0 + 1024],
                                  in_=xt_d.ap()[:, c0:c0 + 1024])

            # ---- A^T = M^T Xq^T (scores stationary), fp16 ----
            for h in range(n_rows // 512):
                pa = psB.tile([128, 2, 512], F32, tag="psB",
                              name=f"paT{h}")[:, 0, :]
                nc.tensor.matmul(pa[:], lhsT=m_t[:],
                                 rhs=xT[:, h * 512:(h + 1) * 512],
                                 start=True, stop=True)
                if h == 0:
                    # first 128 cols first: unblocks est(0) + sub-block 0
                    nc.vector.tensor_copy(aT[:, 0:128], pa[0:128, 0:128])
                    nc.vector.tensor_copy(aT[:, 128:512], pa[:, 128:512])
                else:
                    nc.vector.tensor_copy(aT[:, h * 512:(h + 1) * 512], pa[:])

            # ---- V chunks directly keys-on-partition (bf16); emitted in
            # slices from the main loop (s=0/1 PV slots) ----
            def v_proj_slice(b0):
                pv = psV.tile([128, 4, D], F32, tag="psV",
                              name=f"pv{b0}")
                for j in range(4):
                    nc.tensor.matmul(
                        pv[:, j, :],
                        lhsT=xT[:, (b0 + j) * D:(b0 + j + 1) * D],
                        rhs=wv_t[:], start=True, stop=True)
                nc.vector.tensor_copy(
                    v_b[:, b0:b0 + 4, :].rearrange("p a b -> p (a b)"),
                    pv[:].rearrange("p a b -> p (a b)"))

            negms, lsums, ptsbs, pbufs, l2s, ppvs = {}, {}, {}, {}, {}, {}
            lall = bigp.tile([128, nsb], F32, tag="lall")
            smm_log, pv_pending = [], []

            def _resolve_pv_deps():
                for ent in pv_pending[:]:
                    inst, tgt = ent
                    if tgt < len(smm_log):
                        add_dep_helper(inst.ins, smm_log[tgt].ins,
                                       sync=False,
                                       reason="pv_after_future_smm")
                        pv_pending.remove(ent)

            def est(s):
                """negm(s) = -(max over candidate keys + MARGIN)."""
                ctx = tc.high_priority(offset=500000)
                ctx.__enter__()
                r0 = s * 128
                pe_ = psE.tile([128, N_CAND], F32, tag="psE")
                nc.tensor.matmul(pe_[:], lhsT=aT[:, r0:r0 + 128],
                                 rhs=xc_t[:], start=True, stop=True)
                nraw = smp.tile([128, 1], F32, tag="nraw")
                nc.vector.tensor_reduce(nraw[:], pe_[:],
                                        axis=AX.X, op=ALU.max, negate=True)
                negm = smp.tile([128, 1], F32, tag="negm")
                nc.vector.tensor_scalar_add(negm[:], nraw[:], -MARGIN)
                negms[s] = negm
                ctx.__exit__(None, None, None)

            def pass2_quarter(s, q):
                ctx = tc.high_priority(offset=500000)
                ctx.__enter__()
                r0 = s * 128
                si = s % spg
                if q == 0:
                    pbufs[s] = pbufp.tile([128, n_tok], BF16, tag="p_s",
                                          name=f"p_s{s}")
                    l2s[s] = smp.tile([128, 4], F32, tag="l2",
                                      name=f"l2_{s}")
                    if si == 0:
                        ptsbs[s // spg] = ptsbp.tile(
                            [128, tch, spg, 128], BF16, tag="ptsb",
                            name=f"ptg{s // spg}")
                ps = pbufs[s]
                pt = ptsbs[s // spg]
                pb = psB.tile([128, 2, 512], F32, tag="psB")
                for cc in range(2):
                    k0 = (q * 2 + cc) * 512
                    mm = nc.tensor.matmul(pb[:, cc, :],
                                          lhsT=aT[:, r0:r0 + 128],
                                          rhs=xT[:, k0:k0 + 512],
                                          start=True, stop=True)
                    if cc == 0:
                        smm_log.append(mm)
                _resolve_pv_deps()
                h0 = q * 1024
                if q < 2 or s >= nsb - 2:
                    # ACT accumulates l for the first half (279ns/instr
                    # accumulator read); DVE row-sums the second half from
                    # P in SBUF off the critical path (engine balance).
                    nc.scalar.activation(
                        ps[:, h0:h0 + 1024],
                        pb[:].rearrange("p a b -> p (a b)"),
                        ACTF.Exp, bias=negms[s][:, 0:1],
                        accum_out=l2s[s][:, q:q + 1])
                else:
                    nc.scalar.activation(
                        ps[:, h0:h0 + 1024],
                        pb[:].rearrange("p a b -> p (a b)"),
                        ACTF.Exp, bias=negms[s][:, 0:1])
                if q % 2 == 1:
                    half = q // 2
                    if s >= nsb - 2:
                        for j4 in range(4):
                            ptp = psV.tile([128, 4, 128], BF16, tag="psV",
                                           name=f"ptp{s}_{half}_{j4}")
                            for jj in range(4):
                                j = half * 16 + j4 * 4 + jj
                                nc.tensor.transpose(
                                    ptp[:, jj, :], ps[:, j * 128:(j + 1) * 128],
                                    id_b[:])
                            nc.vector.tensor_copy(
                                pt[:, half * 16 + j4 * 4:
                                   half * 16 + j4 * 4 + 4, si, :],
                                ptp[:])
                    else:
                        nc.sync.dma_start_transpose(
                            out=pt[:, half * 16:(half + 1) * 16, si, :],
                            in_=ps[:, half * 2048:(half + 1) * 2048])
                if q == 3:
                    lsum = smp.tile([128, 1], F32, tag="lsum")
                    if s >= nsb - 2:
                        # tail sub-blocks: l fully from ACT accum (keeps the
                        # DVE reduce off the drain path)
                        nc.vector.tensor_reduce(lsum[:], l2s[s][:],
                                                axis=AX.X, op=ALU.add)
                    else:
                        ldve = smp.tile([128, 1], F32, tag="ldve")
                        nc.vector.tensor_reduce(ldve[:], ps[:, 2048:4096],
                                                axis=AX.X, op=ALU.add)
                        lacc = smp.tile([128, 1], F32, tag="lacc")
                        nc.vector.tensor_reduce(lacc[:], l2s[s][:, 0:2],
                                                axis=AX.X, op=ALU.add)
                        nc.vector.tensor_tensor(out=lsum[:], in0=lacc[:],
                                                in1=ldve[:], op=ALU.add)
                    nc.vector.reciprocal(lall[:, s:s + 1], lsum[:])
                ctx.__exit__(None, None, None)

            def group_pv(g, j0, j1):
                if j0 == 0:
                    ppvs[g] = psV.tile([D, spg * 128], F32, tag="psV",
                                       name=f"ppv{g}")
                ppv = ppvs[g]
                ptg = ptsbs[g]
                for j in range(j0, j1):
                    pvmm = nc.tensor.matmul(
                        ppv[:],
                        lhsT=v_b[:, j, :],
                        rhs=ptg[:, j, :, :].rearrange("p a b -> p (a b)"),
                        start=(j == 0), stop=(j == tch - 1))
                    pv_pending.append((pvmm, len(smm_log) - 1 + 3))

            def group_out(g):
                rw = spg * 128
                ppv = ppvs[g]
                osb = osbp.tile([D, rw], F32, tag="osb")
                nc.vector.tensor_copy(osb[:], ppv[:])
                nc.sync.dma_start(out=out_d.ap()[g * D:(g + 1) * D, :],
                                  in_=osb[:])
                if g == ngr - 1:
                    nc.sync.dma_start(out=l_d.ap(), in_=lall[:])

            # ACT-paced pipeline: est(s+1) + PV slices fill the TE gaps
            # behind each exp; PV groups of 2 sub-blocks drain progressively.
            # per sub-block: 4 quarters (2 S-matmuls + one 1024-wide exp
            # each, psB double-buffered so the exp chain never stalls);
            # est(s+1) and 4-j PV slices of the draining group slot between
            # quarters to keep the PE fed.
            # PV slice schedule: each group's 32 j-chunks run as four 8-j
            # slices starting a full 2 quarters after its last xbar issues
            # (xbar exec lags its exp by ~3-4us: dispatch + DGE delay +
            # queue), in chunkier bursts that keep the PE p-state up.
            sched = {}
            for g_ in range(ngr - 1):
                b_ = 2 * g_ + 2
                sched.setdefault((b_, 2), []).append((g_, 0, 8))
                sched.setdefault((b_, 3), []).append((g_, 8, 16))
                sched.setdefault((b_ + 1, 0), []).append((g_, 16, 24))
                sched.setdefault((b_ + 1, 1), []).append((g_, 24, 32))
            # last group: half-A comes from PE transposes (no DMA), drain in
            # the final sub-block's back quarters
            sched.setdefault((nsb - 1, 2), []).append((ngr - 1, 0, 8))
            sched.setdefault((nsb - 1, 3), []).append((ngr - 1, 8, 16))

            est(0)
            for s in range(nsb):
                for q in range(4):
                    pass2_quarter(s, q)
                    if q == 0 and s + 1 < nsb:
                        est(s + 1)
                    for g_, j0_, j1_ in sched.get((s, q), []):
                        group_pv(g_, j0_, j1_)
                    if s < 2:
                        v_proj_slice((s * 4 + q) * 4)
                if s % 2 == 1 and s >= 3:
                    group_out((s - 3) // 2)
            group_pv(ngr - 1, 16, 32)
            group_out(ngr - 1)

            rep_ctx.__exit__(None, None, None)
    nc.compile()
    return nc


def _get_nc():
    if "nc" not in _NC_CACHE:
        _NC_CACHE["nc"] = _build_attn()
    return _NC_CACHE["nc"]


def _in_maps(input_embeddings, w_query, w_key, w_value):
    d = input_embeddings.shape[-1]
    m_fused = ((w_query.astype(np.float64) @ w_key.astype(np.float64).T)
               / np.sqrt(d)).astype(np.float16)
    wv16 = w_value.astype(np.float16)
    shards_per_b = N_CORES // B_SZ
    maps = []
    for c in range(N_CORES):
        b, s = divmod(c, shards_per_b)
        xb = input_embeddings[b]
        xt_full = xb.astype(np.float16).T                    # [128, 4096]
        # key permutation: this core's own query rows first, so the
        # query slice is xt[:, :N_ROWS] (softmax/PV are key-perm invariant)
        q0 = s * N_ROWS
        xt = np.ascontiguousarray(np.concatenate(
            [xt_full[:, q0:q0 + N_ROWS], xt_full[:, :q0],
             xt_full[:, q0 + N_ROWS:]], axis=1))
        u = xb.sum(axis=1)
        idx = np.argsort(u)
        cand = np.concatenate([idx[:N_CAND // 2], idx[-N_CAND // 2:]])
        xc = np.ascontiguousarray(xt_full[:, cand])          # [128, 32]
        maps.append({"xt": xt, "m": m_fused, "wv": wv16, "xc": xc})
    return maps


def kernel(input_embeddings, token_attention_masks_source=None,
           token_attention_masks_target=None, encoder_output_embedding=None,
           w_query=None, w_key=None, w_value=None, **_unused):
    """Full inputs in, full output out. Runs on 8 NeuronCores (SPMD)."""
    input_embeddings = np.asarray(input_embeddings, dtype=np.float32)
    b_sz, s_src, d = input_embeddings.shape
    assert (b_sz, s_src, d) == (B_SZ, S_SRC, D), "kernel compiled for 2x4096x128"

    maps = _in_maps(input_embeddings,
                    np.asarray(w_query, dtype=np.float32),
                    np.asarray(w_key, dtype=np.float32),
                    np.asarray(w_value, dtype=np.float32))
    res = run_bass_kernel_spmd(_get_nc(), maps, list(range(N_CORES)))
    out = np.empty((B_SZ, S_SRC, D), np.float32)
    shards_per_b = N_CORES // B_SZ
    for c in range(N_CORES):
        b, s = divmod(c, shards_per_b)
        ot = res.results[c]["out"].astype(np.float32).reshape(4, D, 256)
        lr = res.results[c]["lrec"].astype(np.float32)  # [128, 8]
        rows = ot.transpose(0, 2, 1).reshape(N_ROWS, D)
        lrec = lr.T.reshape(N_ROWS, 1)
        out[b, s * N_ROWS:(s + 1) * N_ROWS] = rows * lrec
    return out
